# revision 1
# baseline (speedup 1.0000x reference)
# Multi-head attention (B=2, T=2048, C=768, H=12, D=64) on 8 NeuronCores.
#
# Sharding: core i handles batch b = i // 4 and head group g = i % 4
# (3 heads each).  Each core computes, for its batch slice x_b [T, C]:
#   q/k/v = x_b @ w{q,k,v}[:, g*192:(g+1)*192]          (3 local heads)
#   rope + rmsnorm on q, k; full (non-causal) softmax attention per head
#   partial = attn_out @ wproj[g*192:(g+1)*192, :]       -> [T, C]
# The host sums the 4 partials per batch to form the full output.
#
# Single-core layout strategy (matmul inputs bf16, fp32 accumulation):
#   xT   [128, 6, 2048]  x_b transposed (feature on partition), via PE
#   q/k = xT_chunk.T @ [wq|wk] (fused N=384), v separate (N=192)
#   rope/rmsnorm in [token, head*64] layout (fp32, DVE + ACT Ln/Exp)
#   qT01/kT01 [128, 2048]: rows 0:64 = head0 (d-major), rows 64:128 = head1
#   qT22/kT22 [128, 2048]: head2 duplicated in both halves
#     -> K=64 score matmuls packed two-per-window via PE row tiling
#   scores s^T [tk,tq] = kT.T @ qT ; p = exp(s/8) (ACT, bf16 out, fused
#     across the two heads' psum banks); AV: yplus += [v | 1].T @ p
#   softmax denom = ones-row of yplus; 1/denom on DVE reciprocal,
#   gpsimd partition-broadcast, DVE multiply -> yT (d on partition)
#   proj: out = yT.T @ wp slices, accumulated over the 192 local dims.

import numpy as np
from contextlib import ExitStack

import concourse.hw_specs as _hw_specs
from concourse import mybir

AF = mybir.ActivationFunctionType
ALU = mybir.AluOpType

# Keep Exp/Ln in exactly one ACT table set so bacc's greedy set selection
# never bounces between table sets (each bounce is a ~1.3us table DMA).
if not getattr(_hw_specs, "_mha_act_patch", False):
    _orig_gat = _hw_specs.get_activation_tables

    def _gat_one_exp_ln_set(arch):
        tabs = _orig_gat(arch)
        for name, s in tabs.items():
            if name != "natural_log_exp_and_others":
                s.discard(AF.Exp)
                s.discard(AF.Ln)
        return tabs

    _hw_specs.get_activation_tables = _gat_one_exp_ln_set
    _hw_specs._mha_act_patch = True

import concourse.bass as bass          # noqa: E402
import concourse.tile as tile          # noqa: E402
from concourse import bacc             # noqa: E402
bacc.get_activation_tables = _hw_specs.get_activation_tables
from concourse.bass import ts          # noqa: E402
from concourse.bass_utils import run_bass_kernel_spmd  # noqa: E402
from concourse.masks import make_identity              # noqa: E402

F32 = mybir.dt.float32
BF16 = mybir.dt.bfloat16

T = 2048
C = 768
HL = 3          # heads per core
D = 64
NG = HL * D     # 192, per-core qkv width
NT = T // 128   # 16 token tiles
KC = C // 128   # 6 contraction chunks
TQB = 512       # tq block
NTQ = T // TQB  # 4


def build_kernel(tc, ctx, x, cos, sin, wq, wk, wv, wp, y):
    nc = tc.nc

    const = ctx.enter_context(tc.tile_pool(name="const", bufs=1))
    big = ctx.enter_context(tc.tile_pool(name="big", bufs=1))

    identF = const.tile([128, 128], F32, tag="identF")
    make_identity(nc, identF)
    identB = const.tile([128, 128], BF16, tag="identB")
    make_identity(nc, identB)

    # ---- weights / cos / sin: DMA fp32 staging, cast to bf16 ----
    # wq and wk fused side by side: wqk[:, c, 0:192]=wq, [:, c, 192:384]=wk
    wqk = big.tile([128, KC, 2 * NG], BF16, tag="wqk")
    for i, ap in ((0, wq), (1, wk)):
        st = const.tile([128, KC, NG], F32, tag=f"st_w{i}", name=f"st_w{i}")
        nc.sync.dma_start(out=st, in_=ap.rearrange("(c p) n -> p c n", p=128))
        nc.vector.tensor_copy(wqk[:, :, ts(i, NG)], st)
    wv_st = const.tile([128, KC, NG], F32, tag="wv_st")
    nc.sync.dma_start(out=wv_st, in_=wv.rearrange("(c p) n -> p c n", p=128))
    wv_bf = big.tile([128, KC, NG], BF16, tag="wv_bf")
    nc.vector.tensor_copy(wv_bf, wv_st)

    wpa_st = const.tile([128, C], F32, tag="wpa_st")
    nc.sync.dma_start(out=wpa_st, in_=wp[0:128, :])
    wpa = big.tile([128, C], BF16, tag="wpa")
    nc.vector.tensor_copy(wpa, wpa_st)
    wpb_st = const.tile([64, C], F32, tag="wpb_st")
    nc.sync.dma_start(out=wpb_st, in_=wp[128:192, :])
    wpb = big.tile([64, C], BF16, tag="wpb")
    nc.vector.tensor_copy(wpb, wpb_st)

    cos_sb = big.tile([128, NT, 32], F32, tag="cos")
    nc.sync.dma_start(out=cos_sb, in_=cos.rearrange("(t p) d -> p t d", p=128))
    sin_sb = big.tile([128, NT, 32], F32, tag="sin")
    nc.sync.dma_start(out=sin_sb, in_=sin.rearrange("(t p) d -> p t d", p=128))

    # ---- persistent big tensors ----
    xT = big.tile([128, KC, T], BF16, tag="xT")
    qT01 = big.tile([128, T], BF16, tag="qT01")
    kT01 = big.tile([128, T], BF16, tag="kT01")
    qT22 = big.tile([128, T], BF16, tag="qT22")
    kT22 = big.tile([128, T], BF16, tag="kT22")
    yTa = big.tile([128, T], BF16, tag="yTa")   # rows 0:64 head0, 64:128 head1
    yTb = big.tile([64, T], BF16, tag="yTb")    # head2
    v_all = big.tile([128, NT, HL, 65], BF16, tag="v_all")
    q_stash = big.tile([128, NT, HL, 64], F32, tag="q_stash")
    nc.gpsimd.memset(v_all[:, :, :, 64:65], 1.0)

    work = ctx.enter_context(tc.tile_pool(name="work", bufs=4))

    def qk_side(src, t, d01, d22, psCp, pstag, d01_act, inv_pre=None):
        """rope + rmsnorm + head transposes for one of q/k.
        src: [128, HL, 64] fp32 SBUF AP for token tile t."""
        cos_t = cos_sb[:, t, :]
        sin_t = sin_sb[:, t, :]
        cos_b = bass.AP(tensor=cos_t.tensor, offset=cos_t.offset,
                        ap=[cos_t.ap[0], [0, HL], [0, 2], cos_t.ap[1]])
        sin_b = bass.AP(tensor=sin_t.tensor, offset=sin_t.offset,
                        ap=[sin_t.ap[0], [0, HL], [0, 2], sin_t.ap[1]])
        srcu = src.rearrange("p h (u d) -> p h u d", u=2)
        tcc = work.tile([128, HL, 2, 32], F32, tag="tcc",
                        name=f"tcc_{t}_{d01.name}")
        tss = work.tile([128, HL, 2, 32], F32, tag="tss",
                        name=f"tss_{t}_{d01.name}")
        qr = work.tile([128, HL, 64], F32, tag="qr", name=f"qr_{t}_{d01.name}")
        nc.vector.tensor_mul(tcc, srcu, cos_b)
        nc.vector.tensor_mul(tss, srcu, sin_b)
        nc.vector.tensor_add(qr[:, :, 0:32], tcc[:, :, 0, :], tss[:, :, 1, :])
        nc.vector.tensor_sub(qr[:, :, 32:64], tcc[:, :, 1, :],
                             tss[:, :, 0, :])

        # rmsnorm: inv = exp(-0.5*ln(sum(x^2)/64 + eps))
        if inv_pre is None:
            sq = work.tile([128, HL, 64], F32, tag="sq",
                           name=f"sq_{t}_{d01.name}")
            nc.vector.tensor_mul(sq, qr, qr)
            ms = work.tile([128, HL], F32, tag="ms",
                           name=f"ms_{t}_{d01.name}")
            nc.vector.reduce_sum(ms, sq, axis=mybir.AxisListType.X)
            mse = work.tile([128, HL], F32, tag="mse",
                            name=f"mse_{t}_{d01.name}")
            nc.vector.tensor_scalar_add(mse, ms, 64.0e-6)
            lms = work.tile([128, HL], F32, tag="lms",
                            name=f"lms_{t}_{d01.name}")
            nc.scalar.activation(lms, mse, AF.Ln, scale=1.0 / 64.0)
            inv = work.tile([128, HL], F32, tag="inv",
                            name=f"inv_{t}_{d01.name}")
            nc.scalar.activation(inv, lms, AF.Exp, scale=-0.5)
        else:
            inv = inv_pre
        qhat = work.tile([128, HL, 64], BF16, tag="qhat",
                         name=f"qhat_{t}_{d01.name}")
        for h in range(HL):
            nc.vector.tensor_scalar_mul(qhat[:, h, :], qr[:, h, :],
                                        inv[:, h:h + 1])

        # transpose heads: [tok, h*64] -> [d, tok]
        qhf = qhat.rearrange("p h d -> p (h d)")
        tp1 = psCp.tile([128, 2, 128], BF16, tag=pstag,
                        name=f"tp_{t}_{d01.name}")
        nc.tensor.transpose(tp1[:, 0, :], qhf[:, 0:128], identB)
        nc.tensor.transpose(tp1[0:64, 1, :], qhf[:, 128:192], identB)
        nc.tensor.transpose(tp1[64:128, 1, :], qhf[:, 128:192],
                            identB, tile_position=(0, 64))
        if d01_act:
            nc.scalar.copy(d01[:, ts(t, 128)], tp1[:, 0, :])
        else:
            nc.vector.tensor_copy(d01[:, ts(t, 128)], tp1[:, 0, :])
        nc.vector.tensor_copy(d22[:, ts(t, 128)], tp1[:, 1, :])

    # ===== pass 1: x load/transpose, QKV matmuls, k/v processing =====
    xpool = ctx.enter_context(tc.tile_pool(name="xin", bufs=3))
    with tc.tile_pool(name="psA", bufs=2, space="PSUM") as psA, \
         tc.tile_pool(name="psQK", bufs=2, space="PSUM") as psQK, \
         tc.tile_pool(name="psV", bufs=2, space="PSUM") as psV, \
         tc.tile_pool(name="psC", bufs=2, space="PSUM") as psC:
        for t in range(NT):
            x_t = xpool.tile([128, C], F32, tag="x_t")
            nc.sync.dma_start(out=x_t, in_=x[ts(t, 128), :])

            # transpose x tile -> xT (bf16 cast on the psum->sbuf copy)
            tpa = psA.tile([128, 4, 128], F32, tag="tpa", bufs=2)
            tpb = psA.tile([128, 2, 128], F32, tag="tpa", bufs=2,
                           name=f"tpb_{t}")
            for c in range(KC):
                dst = tpa[:, c, :] if c < 4 else tpb[:, c - 4, :]
                nc.tensor.transpose(dst, x_t[:, ts(c, 128)], identF)
            nc.scalar.copy(xT[:, 0:4, ts(t, 128)], tpa)
            nc.scalar.copy(xT[:, 4:6, ts(t, 128)], tpb)

            # QKV matmuls: q and k fused (N=384), v separate
            qk_ps = psQK.tile([128, 2, NG], F32, tag="qk_ps")
            v_ps = psV.tile([128, NG], F32, tag="v_ps")
            qk_f = qk_ps.rearrange("p a n -> p (a n)")
            for ci in range(KC):
                nc.tensor.matmul(qk_f, lhsT=xT[:, ci, ts(t, 128)],
                                 rhs=wqk[:, ci, :],
                                 start=(ci == 0), stop=(ci == KC - 1))
            for ci in range(KC):
                nc.tensor.matmul(v_ps, lhsT=xT[:, ci, ts(t, 128)],
                                 rhs=wv_bf[:, ci, :],
                                 start=(ci == 0), stop=(ci == KC - 1))

            # v: psum -> sbuf bf16 (ones column pre-set)
            v_ps3 = v_ps.rearrange("p (h d) -> p h d", h=HL)
            nc.scalar.copy(v_all[:, t, :, 0:64], v_ps3)

            # stash raw q for pass 2; stage k for immediate processing
            qk_ps4 = qk_ps.rearrange("p a (h d) -> p a h d", h=HL)
            nc.vector.tensor_copy(q_stash[:, t, :, :], qk_ps4[:, 0, :, :])
            qk_side(qk_ps4[:, 1, :, :], t, kT01, kT22, psC, "tpk",
                    d01_act=True)

    # ===== pass 2 + attention + projection, per tq chunk =====
    ppool = ctx.enter_context(tc.tile_pool(name="ppool", bufs=6))
    dn = ctx.enter_context(tc.tile_pool(name="dn", bufs=4))
    opool = ctx.enter_context(tc.tile_pool(name="opool", bufs=4))
    with tc.tile_pool(name="sps", bufs=2, space="PSUM") as sps, \
         tc.tile_pool(name="psY", bufs=1, space="PSUM") as psY, \
         tc.tile_pool(name="psP", bufs=1, space="PSUM") as psP:
        def q_chunk(tq):
            for tt in range(4):
                t = 4 * tq + tt
                qk_side(q_stash[:, t, :, :], t, qT01, qT22, psP, "aux",
                        d01_act=False)

        q_chunk(0)
        for tq in range(NTQ):
            tqs = ts(tq, TQB)
            # prepare the NEXT chunk's qT while this chunk's attention
            # (ACT-bound) runs; its psum slot frees early in this chunk.
            if tq + 1 < NTQ:
                q_chunk(tq + 1)

            yp = [psY.tile([65, TQB], F32, tag=f"yp{h}", bufs=1,
                           name=f"yp{h}_{tq}")
                  for h in range(HL)]
            for tkp in range(NT // 2):
                tk0, tk1 = 2 * tkp, 2 * tkp + 1
                # three 2-bank score tiles per tk pair, one fused exp each;
                # each tile's two matmuls use opposite PE row halves.
                for tag, mm in (
                    ("sA", ((kT01, qT01, slice(0, 64), tk0, None, 0),
                            (kT01, qT01, slice(64, 128), tk0, (64, 0), 1))),
                    ("sB", ((kT01, qT01, slice(0, 64), tk1, None, 0),
                            (kT01, qT01, slice(64, 128), tk1, (64, 0), 1))),
                    ("sC", ((kT22, qT22, slice(0, 64), tk0, None, 2),
                            (kT22, qT22, slice(64, 128), tk1, (64, 0), 2))),
                ):
                    s = sps.tile([128, 2, TQB], F32, tag="s",
                                 name=f"{tag}_{tq}_{tkp}")
                    for i, (kT, qT, half, tk, pos, _h) in enumerate(mm):
                        nc.tensor.matmul(s[:, i, :],
                                         lhsT=kT[half, ts(tk, 128)],
                                         rhs=qT[half, tqs],
                                         start=True, stop=True,
                                         tile_position=pos)
                    p = ppool.tile([128, 2, TQB], BF16, tag="p",
                                   name=f"p{tag}_{tq}_{tkp}")
                    nc.scalar.activation(p.rearrange("p a n -> p (a n)"),
                                         s.rearrange("p a n -> p (a n)"),
                                         AF.Exp, scale=0.125)
                    for i, (kT, qT, half, tk, pos, h) in enumerate(mm):
                        nc.tensor.matmul(yp[h], lhsT=v_all[:, tk, h, :],
                                         rhs=p[:, i, :],
                                         start=(tk == 0),
                                         stop=(tk == NT - 1))

            # normalize: row 64 of yp is the softmax denominator
            for h in range(HL):
                rec = dn.tile([1, TQB], F32, tag="rec", name=f"rec{h}_{tq}")
                nc.vector.reciprocal(rec, yp[h][64:65, :])
                rb = dn.tile([64, TQB], F32, tag="rb", name=f"rb{h}_{tq}")
                nc.gpsimd.partition_broadcast(rb, rec)
                if h == 0:
                    dst = yTa[0:64, tqs]
                elif h == 1:
                    dst = yTa[64:128, tqs]
                else:
                    dst = yTb[:, tqs]
                nc.vector.tensor_mul(dst, yp[h][0:64, :], rb)

            # projection for this tq chunk's 4 token tiles
            for tt in range(4):
                t = 4 * tq + tt
                o_sb = opool.tile([128, C], F32, tag="o_sb",
                                  name=f"o_sb_{t}")
                for nh in range(2):
                    nsl = ts(nh, 384)
                    pp = psP.tile([128, 384], F32, tag="aux",
                                  name=f"pp_{t}_{nh}")
                    nc.tensor.matmul(pp, lhsT=yTa[:, ts(t, 128)],
                                     rhs=wpa[:, nsl], start=True, stop=False)
                    nc.tensor.matmul(pp, lhsT=yTb[:, ts(t, 128)],
                                     rhs=wpb[:, nsl], start=False, stop=True)
                    nc.vector.tensor_copy(o_sb[:, nsl], pp)
                nc.sync.dma_start(out=y[ts(t, 128), :], in_=o_sb)


def build_nc(reps=1):
    nc = bacc.Bacc("TRN2", target_bir_lowering=False, debug=False,
                   num_devices=8)
    x = nc.dram_tensor("x", [T, C], F32, kind="ExternalInput").ap()
    cos = nc.dram_tensor("cos", [T, 32], F32, kind="ExternalInput").ap()
    sin = nc.dram_tensor("sin", [T, 32], F32, kind="ExternalInput").ap()
    wq = nc.dram_tensor("wq", [C, NG], F32, kind="ExternalInput").ap()
    wk = nc.dram_tensor("wk", [C, NG], F32, kind="ExternalInput").ap()
    wv = nc.dram_tensor("wv", [C, NG], F32, kind="ExternalInput").ap()
    wp = nc.dram_tensor("wp", [NG, C], F32, kind="ExternalInput").ap()
    y = nc.dram_tensor("y", [T, C], F32, kind="ExternalOutput").ap()
    with tile.TileContext(nc) as tc:
        for _ in range(reps):
            with ExitStack() as ctx:
                build_kernel(tc, ctx, x, cos, sin, wq, wk, wv, wp, y)
    nc.compile()
    return nc


def make_in_maps(x, cos, sin, wq, wk, wv, wproj):
    cos2 = np.ascontiguousarray(np.asarray(cos, np.float32).reshape(T, 32))
    sin2 = np.ascontiguousarray(np.asarray(sin, np.float32).reshape(T, 32))
    in_maps = []
    for cid in range(8):
        b, g = divmod(cid, 4)
        sl = slice(g * NG, (g + 1) * NG)
        in_maps.append({
            "x": np.ascontiguousarray(np.asarray(x, np.float32)[b]),
            "cos": cos2,
            "sin": sin2,
            "wq": np.ascontiguousarray(np.asarray(wq, np.float32)[:, sl]),
            "wk": np.ascontiguousarray(np.asarray(wk, np.float32)[:, sl]),
            "wv": np.ascontiguousarray(np.asarray(wv, np.float32)[:, sl]),
            "wp": np.ascontiguousarray(np.asarray(wproj, np.float32)[sl, :]),
        })
    return in_maps


_NC = None


def kernel(x, cos, sin, wq, wk, wv, wproj):
    global _NC
    if _NC is None:
        _NC = build_nc()
    in_maps = make_in_maps(x, cos, sin, wq, wk, wv, wproj)
    res = run_bass_kernel_spmd(_NC, in_maps, list(range(8)))
    outs = [r["y"].astype(np.float64) for r in res.results]
    y0 = outs[0] + outs[1] + outs[2] + outs[3]
    y1 = outs[4] + outs[5] + outs[6] + outs[7]
    return np.stack([y0, y1], axis=0).astype(np.float32)


if __name__ == "__main__":
    rng = np.random.default_rng(0)
    ins = {
        "x": rng.standard_normal((2, T, C), dtype=np.float32),
        "cos": rng.random((T, 1, 32), dtype=np.float32),
        "sin": rng.random((T, 1, 32), dtype=np.float32),
        "wq": rng.standard_normal((C, C), dtype=np.float32) / np.sqrt(C),
        "wk": rng.standard_normal((C, C), dtype=np.float32) / np.sqrt(C),
        "wv": rng.standard_normal((C, C), dtype=np.float32) / np.sqrt(C),
        "wproj": rng.standard_normal((C, C), dtype=np.float32) / np.sqrt(C),
    }
    out = kernel(**ins)
    print(out.shape, out.dtype, np.abs(out).max())

